# revision 1
# baseline (speedup 1.0000x reference)
"""DSA sparse attention (context-parallel variant) for Trainium2 via Bass/Tile.

Dense-rewrite algorithm (mathematically identical to the reference):
  w[s,t] = exp(sc[s,t])*ts[s,t] / sum_t' exp(sc)*ts   (softmax->*ts->renorm collapses)
  TS[s,j] = sum_t ts[s,t]*[idx[s,t]==j]  -> scatter of input values (dup-safe)
  E[s,j]  = TS[s,j]*exp(scale*S[s,j]),  S = Q K^T (dense)
  O       = (E @ V) / rowsum(E)
Everything is computed in transposed layout (kv on partitions); O comes out
natural via E^T-stationary matmuls; rowsum(E) falls out of a ones-column
appended to V.

V2 layout/scheduling notes:
  - host pre-converts q (pre-scaled), k, v, identity, scatter values to bf16:
    halves input DMA, removes DVE convert passes, enables FWL on PE weight
    loads (transposes + matmul stationary operands).
  - phases run g-major: (h0,g0) (h1,g0) (h0,g1) (h1,g1), so group-1 scatters
    (Pool) overlap the first two compute phases.
  - per phase, the S^T matmuls of this phase are WOVEN with the E^T V matmuls
    of the previous phase so the PE alternates between ACT-gated S work and
    dependency-free EV work; ACT (exp) never starves and PE never stalls.
"""

import sys

sys.path.insert(0, "/opt/trn_rl_repo")

import numpy as np

import concourse.bass as bass
import concourse.bacc as bacc
import concourse.mybir as mybir
import concourse.tile as tile
from concourse.vector_clock import ScopedClock

# ---------------------------------------------------------------------------
# Patch: this walrus build encodes at most ONE sync-wait on a CTRL NO_STRUCT
# instruction; TileContext's tail drain carries one wait per live proc.  Split
# the waits across a chain of single-wait drains.
# ---------------------------------------------------------------------------


def _patched_drain_and_barrier(self, tick_clock, wait_clock):
    drain_inst = self.nc.sync.drain()
    wait_clock.add_sem_waits(
        drain_inst.ins, ScopedClock({None: tick_clock.global_clock})
    )
    si = drain_inst.ins.sync_info
    if si is not None and len(si.on_wait) > 1:
        waits = list(si.on_wait)
        drain_inst.ins.sync_info = mybir.SyncInfo(
            on_wait=waits[:1], on_update=list(si.on_update)
        )
        for i in range(1, len(waits)):
            extra = self.nc.sync.drain()
            extra.ins.sync_info = mybir.SyncInfo(on_wait=[waits[i]], on_update=[])
    self.nc.all_engine_barrier()
    assert self.sems is not None
    popped = self.nc._tile_sem_poison_stack.pop()
    assert popped is self._sem_poison
    self.nc.clear_and_free_semaphores(list(self.sems.allocated().values()))
    self.nc.all_engine_barrier()


tile.TileContext._drain_and_barrier = _patched_drain_and_barrier

FP = mybir.dt.float32
BF = mybir.dt.bfloat16
I16 = mybir.dt.int16


class Cfg:
    def __init__(self, HPC=2, SQ=1024, SKV=4096, D=128, TOPK=64):
        self.HPC = HPC  # heads per core
        self.SQ = SQ
        self.SKV = SKV
        self.D = D
        self.TOPK = TOPK
        self.NKV = SKV // 128  # kv chunks of 128
        self.NSB = SQ // 128  # query blocks of 128
        self.SHALF = 512  # scatter group width (s-dim per group)
        self.scale = float(D) ** -0.5
        self.ablate = None  # timing-diagnosis only: 'noev' | 'noexpmul' | 'sonly' | 'noscat' 


# ---------------------------------------------------------------------------
# Host-side index preprocessing: invert (query -> kv rows) into
# (kv row -> queries) lists, merging duplicate (s, j) pairs.
# ---------------------------------------------------------------------------


def host_prep_scatter(topk_indices, topk_scores, cfg):
    """Invert (query -> kv rows) into (kv row -> queries), merging duplicate
    (s, j) pairs by summing their scores (the same reduction the reference's
    gather+softmax performs).  Emitted per (kv-chunk, s-group)."""
    SQ, TOPK, SKV = cfg.SQ, cfg.TOPK, cfg.SKV
    HALF = cfg.SHALF
    NH = SQ // HALF
    s_arr = np.repeat(np.arange(SQ, dtype=np.int64), TOPK)
    j_arr = topk_indices.reshape(-1).astype(np.int64)
    v_arr = topk_scores.reshape(-1).astype(np.float32)

    sj = s_arr * SKV + j_arr
    uniq, inv = np.unique(sj, return_inverse=True)
    vals = np.zeros(len(uniq), dtype=np.float32)
    np.add.at(vals, inv, v_arr)
    sp = (uniq // SKV).astype(np.int64)
    jp = (uniq % SKV).astype(np.int64)

    idx_list, val_list = [], []
    for hf in range(NH):
        m = (sp >= hf * HALF) & (sp < (hf + 1) * HALF)
        sh, jh, vh = sp[m] - hf * HALF, jp[m], vals[m]
        perm2 = np.argsort(jh, kind="stable")
        jps = jh[perm2]
        ng = np.r_[True, np.diff(jps) != 0]
        gs = np.maximum.accumulate(np.where(ng, np.arange(len(jps)), 0))
        slot_sorted = np.arange(len(jps)) - gs
        slot = np.empty(len(jh), dtype=np.int64)
        slot[perm2] = slot_sorted
        nmax = int(slot.max()) + 1 if len(slot) else 1
        nmax = max(2, (nmax + 1) // 2 * 2)
        idx16 = np.full((cfg.NKV, 128, nmax), -1, dtype=np.int16)
        valsd = np.zeros((cfg.NKV, 128, nmax), dtype=np.float32)
        idx16[jh // 128, jh % 128, slot] = sh.astype(np.int16)
        valsd[jh // 128, jh % 128, slot] = vh
        idx_list.append(idx16)
        val_list.append(valsd)
    return idx_list, val_list


# ---------------------------------------------------------------------------
# Program builder
# ---------------------------------------------------------------------------


def build_program(cfg, nmaxs, reps=1):
    nc = bacc.Bacc("TRN2", debug=False)
    HPC, SQ, SKV, D, NKV = cfg.HPC, cfg.SQ, cfg.SKV, cfg.D, cfg.NKV
    NGRP = SQ // cfg.SHALF

    q = nc.dram_tensor("q", [HPC, SQ, D], BF, kind="ExternalInput").ap()
    k = nc.dram_tensor("k", [HPC, SKV, D], BF, kind="ExternalInput").ap()
    v = nc.dram_tensor("v", [HPC, SKV, D], BF, kind="ExternalInput").ap()
    ident = nc.dram_tensor("ident", [128, 128], BF, kind="ExternalInput").ap()
    sc_idx = [
        nc.dram_tensor(f"sc_idx_{p}", [NKV, 128, nmaxs[p]], I16, kind="ExternalInput").ap()
        for p in range(NGRP)
    ]
    sc_val = [
        nc.dram_tensor(f"sc_val_{p}", [NKV, 128, nmaxs[p]], BF, kind="ExternalInput").ap()
        for p in range(NGRP)
    ]
    out = nc.dram_tensor("out", [HPC, SQ, D], FP, kind="ExternalOutput").ap()

    with tile.TileContext(nc) as tc:
        import contextlib

        ctx = contextlib.ExitStack()
        with ctx:
            const_pool = ctx.enter_context(tc.tile_pool(name="const", bufs=1))
            tst_pool = ctx.enter_context(tc.tile_pool(name="tst", bufs=2))
            stage_pool = ctx.enter_context(tc.tile_pool(name="stage", bufs=2))
            ktr_pool = ctx.enter_context(tc.tile_pool(name="ktr", bufs=2))
            et_pool = ctx.enter_context(tc.tile_pool(name="et", bufs=2))
            sc_pool = ctx.enter_context(tc.tile_pool(name="scst", bufs=2))
            small_pool = ctx.enter_context(tc.tile_pool(name="small", bufs=4))
            out_pool = ctx.enter_context(tc.tile_pool(name="outp", bufs=4))
            s_psum = ctx.enter_context(tc.tile_pool(name="sps", bufs=2, space="PSUM"))
            # transposes (prep) and EV accumulators (compute) are active at
            # disjoint times; share one 4-slot single-bank pool between them
            mix_psum = ctx.enter_context(tc.tile_pool(name="mix", bufs=4, space="PSUM"))

            identity = const_pool.tile([128, 128], BF, tag="ident")
            nc.sync.dma_start(identity[:], ident[:])

            def _body(_iv=None):
                _build_body(
                    nc, tc, cfg, nmaxs, q, k, v, sc_idx, sc_val, out, identity,
                    tst_pool, stage_pool, ktr_pool, et_pool, sc_pool,
                    small_pool, out_pool, s_psum, mix_psum,
                )

            if reps == 1:
                _body()
            else:
                with tc.For_i(
                    0, reps, 1,
                    hint_engines=(
                        mybir.EngineType.PE,
                        mybir.EngineType.DVE,
                        mybir.EngineType.Activation,
                        mybir.EngineType.Pool,
                        mybir.EngineType.SP,
                    ),
                ):
                    _body()

    nc.compile()
    return nc


def _build_body(nc, tc, cfg, nmaxs, q, k, v, sc_idx, sc_val, out, identity,
                tst_pool, stage_pool, ktr_pool, et_pool, sc_pool,
                small_pool, out_pool, s_psum, mix_psum):
    HPC, SQ, SKV, D, NKV = cfg.HPC, cfg.SQ, cfg.SKV, cfg.D, cfg.NKV
    SGRP = cfg.SHALF
    NGRP = SQ // SGRP
    NSBG = SGRP // 128  # s-blocks per group (4)
    STG = 16            # kv-chunks per staging DMA

    # ---------------- scatter-group index/value DMAs + Pool scatters --------
    tsts = []
    for g in range(NGRP):
        nm = nmaxs[g]
        tst = tst_pool.tile([128, NKV, SGRP], BF, tag="tst")
        idx_t = sc_pool.tile([128, NKV, nm], I16, tag="sidx")
        val_t = sc_pool.tile([128, NKV, nm], BF, tag="sval")
        nc.sync.dma_start(idx_t[:], sc_idx[g].rearrange("n p m -> p n m"))
        nc.sync.dma_start(val_t[:], sc_val[g].rearrange("n p m -> p n m"))
        for J in range(NKV if cfg.ablate != "noscat" else 0):
            nc.gpsimd.local_scatter(
                tst[:, J, :],
                val_t[:, J, :], idx_t[:, J, :],
                channels=128, num_elems=SGRP, num_idxs=nm,
            )
        tsts.append(tst)

    # ---------------- per-head staged loads + transposes --------------------
    def _prep_q(h):
        """qtr[d, s] (pre-scaled bf16) via PE transposes."""
        qtr = ktr_pool.tile([128, SQ], BF, tag="qtr")
        qst = stage_pool.tile([128, cfg.NSB, D], BF, tag="qst")
        nc.sync.dma_start(qst[:], q[h].rearrange("(n p) d -> p n d", p=128))

        def emit_block(b):
            trp = mix_psum.tile([128, 256], BF, tag="mix")
            nc.tensor.transpose(trp[:, 0:128], qst[:, 2 * b, :], identity[:])
            nc.tensor.transpose(trp[:, 128:256], qst[:, 2 * b + 1, :], identity[:])
            nc.vector.tensor_copy(qtr[:, 2 * b * 128 : (2 * b + 2) * 128], trp[:])

        return qtr, [lambda b=b: emit_block(b) for b in range(cfg.NSB // 2)]

    def _prep_k(h):
        """ktr[d, kv] bf16 via PE transposes; returns (tile, emit thunks).
        Tiles are created inside the thunks so tile-pool slot rotation order
        matches instruction emission order."""
        ktr = ktr_pool.tile([128, SKV], BF, tag="ktr")
        kview = k[h].rearrange("(n p) d -> p n d", p=128)
        thunks = []
        shared = {}
        for gdma in range(NKV // STG):
            def emit_dma(gdma=gdma):
                kst = stage_pool.tile([128, STG, D], BF, tag="kst")
                shared[gdma] = kst
                nc.sync.dma_start(
                    kst[:], kview[:, gdma * STG : (gdma + 1) * STG, :]
                )

            thunks.append(emit_dma)
            for j in range(0, STG, 2):
                def emit_tr(gdma=gdma, j=j):
                    kst = shared[gdma]
                    J = gdma * STG + j
                    trp = mix_psum.tile([128, 256], BF, tag="mix")
                    nc.tensor.transpose(trp[:, 0:128], kst[:, j, :], identity[:])
                    nc.tensor.transpose(trp[:, 128:256], kst[:, j + 1, :], identity[:])
                    nc.vector.tensor_copy(ktr[:, J * 128 : (J + 2) * 128], trp[:])

                thunks.append(emit_tr)
        return ktr, thunks

    def _prep_v(h):
        """vaug[kv, J, d+1] bf16, DMA'd directly; ones column via memset."""
        vaug = ktr_pool.tile([128, NKV, D + 1], BF, tag="vaug")
        vview = v[h].rearrange("(n p) d -> p n d", p=128)
        for gdma in range(NKV // STG):
            nc.sync.dma_start(
                vaug[:, gdma * STG : (gdma + 1) * STG, 0:D],
                vview[:, gdma * STG : (gdma + 1) * STG, :],
            )
        nc.vector.memset(vaug[:, :, D : D + 1], 1.0)
        return vaug

    # prep head 0 eagerly (first phase needs it); head 1 woven into phase 0
    qtr0, qthunks0 = _prep_q(0)
    for t in qthunks0:
        t()
    ktr0, kthunks0 = _prep_k(0)
    for t in kthunks0:
        t()
    vaug0 = _prep_v(0)
    qtr1, qthunks1 = _prep_q(1)
    ktr1, kthunks1 = _prep_k(1)
    vaug1 = _prep_v(1)
    prep1 = qthunks1 + kthunks1  # emitted inside phase 0's weave
    qtrs, ktrs, vaugs = [qtr0, qtr1], [ktr0, ktr1], [vaug0, vaug1]

    # ---------------- woven compute phases ----------------------------------
    # phase list g-major so group-1 scatters overlap phases 0-1
    phases = [(h, g) for g in range(NGRP) for h in range(HPC)]

    class EvState:
        """Pending EV accumulation for one finished phase.  The four
        s-block accumulation groups advance J-outer/b-inner over four
        rotating single-bank tiles so consecutive matmuls never target the
        same PSUM region (avoids the accumulate read-modify-write stall);
        each block is normalized and stored as soon as its group closes."""

        def __init__(self, h, g, et):
            self.h, self.g, self.et = h, g, et
            self.seq = [(b, J) for J in range(NKV) for b in range(NSBG)]
            self.pos = 0
            self.ops = [None] * NSBG

        def emit(self, n):
            vaug = vaugs[self.h]
            for _ in range(n):
                if self.pos >= len(self.seq):
                    return
                b, J = self.seq[self.pos]
                if J == 0:
                    self.ops[b] = mix_psum.tile(
                        [128, D + 1], FP, tag="mix", name="evacc"
                    )
                nc.tensor.matmul(
                    self.ops[b][:, :],
                    self.et[:, J, b * 128 : (b + 1) * 128],
                    vaug[:, J, :],
                    start=(J == 0), stop=(J == NKV - 1),
                )
                if J == NKV - 1:
                    sb = self.g * NSBG + b
                    recip = small_pool.tile([128, 1], FP, tag="recip")
                    nc.vector.reciprocal(recip[:], self.ops[b][:, D : D + 1])
                    ot = out_pool.tile([128, D], FP, tag="ot")
                    nc.vector.tensor_scalar_mul(ot[:], self.ops[b][:, 0:D], recip[:])
                    nc.sync.dma_start(
                        out[self.h, sb * 128 : (sb + 1) * 128, :], ot[:]
                    )
                self.pos += 1

        def finish(self):
            self.emit(len(self.seq) - self.pos)

    prev = None  # EvState of the phase whose EV is pending
    for pi, (h, g) in enumerate(phases):
        qtr, ktr, tst = qtrs[h], ktrs[h], tsts[g]
        et = et_pool.tile([128, NKV, SGRP], BF, tag="et")
        sl = slice(g * SGRP, (g + 1) * SGRP)
        for Jp in range(NKV // 2):
            J = 2 * Jp
            sp = s_psum.tile([128, 2, SGRP], FP, tag="sps")
            nc.tensor.matmul(
                sp[:, 0, :], ktr[:, J * 128 : (J + 1) * 128], qtr[:, sl],
                start=True, stop=True,
            )
            nc.tensor.matmul(
                sp[:, 1, :], ktr[:, (J + 1) * 128 : (J + 2) * 128], qtr[:, sl],
                start=True, stop=True,
            )
            # weave: EV matmuls of the previous phase, plus head-1 prep
            if prev is not None and Jp % 2 == 1:
                prev.emit(16)
            if pi == 0 and prep1:
                for _ in range(3):
                    if prep1:
                        prep1.pop(0)()
            if cfg.ablate not in ("noexpmul", "sonly"):
                nc.scalar.activation(
                    et[:, J : J + 2, :], sp[:], mybir.ActivationFunctionType.Exp
                )
                nc.vector.tensor_mul(
                    et[:, J : J + 2, :], et[:, J : J + 2, :], tst[:, J : J + 2, :]
                )
        if prev is not None:
            prev.finish()
        if cfg.ablate not in ("noev", "sonly"):
            prev = EvState(h, g, et)

    # tail: EV of the last phase runs unwoven (next rep's prep overlaps it)
    if prev is not None:
        prev.finish()


# ---------------------------------------------------------------------------
# Entry point: full unsharded inputs -> full output.
# Sharding: head-parallel, 2 heads per NeuronCore across 8 cores; the
# topk index/score tensors are shared by all cores.
# ---------------------------------------------------------------------------

_CACHE = {}


def make_in_maps(q, k, v, topk_indices, topk_scores, cfg):
    """Host-side prep: bf16 conversion, q pre-scaling, scatter lists.
    Returns (in_maps, nmaxs)."""
    import ml_dtypes

    bf16 = ml_dtypes.bfloat16
    idx_arrs, val_arrs = host_prep_scatter(
        np.asarray(topk_indices)[0],
        np.asarray(topk_scores, dtype=np.float32)[0],
        cfg,
    )
    nmaxs = tuple(a.shape[-1] for a in idx_arrs)
    qs = (np.asarray(q, dtype=np.float32) * (float(cfg.D) ** -0.5)).astype(bf16)
    kb = np.asarray(k, dtype=np.float32).astype(bf16)
    vb = np.asarray(v, dtype=np.float32).astype(bf16)
    ident = np.eye(128, dtype=bf16)
    val_bf = [a.astype(bf16) for a in val_arrs]
    in_maps = []
    for i in range(8):
        m = {
            "q": np.ascontiguousarray(qs[0, 2 * i : 2 * i + 2]),
            "k": np.ascontiguousarray(kb[0, 2 * i : 2 * i + 2]),
            "v": np.ascontiguousarray(vb[0, 2 * i : 2 * i + 2]),
            "ident": ident,
        }
        for p, (ia, va) in enumerate(zip(idx_arrs, val_bf)):
            m[f"sc_idx_{p}"] = ia
            m[f"sc_val_{p}"] = va
        in_maps.append(m)
    return in_maps, nmaxs


def kernel(q, k, v, topk_indices, topk_scores):
    q = np.asarray(q, dtype=np.float32)
    B, H, SQ, D = q.shape
    SKV = np.asarray(k).shape[2]
    TOPK = np.asarray(topk_indices).shape[-1]
    assert B == 1 and H == 16 and SQ == 1024 and SKV == 4096 and D == 128

    cfg = Cfg(HPC=H // 8, SQ=SQ, SKV=SKV, D=D, TOPK=TOPK)
    in_maps, nmaxs = make_in_maps(q, k, v, topk_indices, topk_scores, cfg)

    nc = _CACHE.get(nmaxs)
    if nc is None:
        nc = build_program(cfg, list(nmaxs), reps=1)
        _CACHE[nmaxs] = nc

    from concourse.bass_utils import run_bass_kernel_spmd

    res = run_bass_kernel_spmd(nc, in_maps, list(range(8)))
    out = np.stack([res.results[i]["out"] for i in range(8)])
    return out.reshape(1, H, SQ, D).astype(np.float32)



# revision 3
# speedup vs baseline: 1.2194x; 1.2194x over previous
"""DSA sparse attention (context-parallel variant) for Trainium2 via Bass/Tile.

Dense-rewrite algorithm (mathematically identical to the reference):
  w[s,t] = exp(sc[s,t])*ts[s,t] / sum_t' exp(sc)*ts   (softmax->*ts->renorm collapses)
  TS[s,j] = sum_t ts[s,t]*[idx[s,t]==j]  -> dense scatter of score values
  E[s,j]  = TS[s,j]*exp(scale*S[s,j]),  S = Q K^T (dense)
  O       = (E @ V) / rowsum(E)
Everything is computed in transposed layout (kv on partitions); O comes out
natural via E^T-stationary matmuls; rowsum(E) falls out of a ones-column
appended to V.

V3 layout/scheduling notes (over V2):
  - host pre-TRANSPOSES q (pre-scaled) and k to [D, S] layout: the on-chip
    PE transposes (80 matmuls) + DVE evacuation copies disappear and the
    q/k DMAs become perfectly contiguous per partition.
  - host pre-builds the DENSE TS table (bf16, [128, NKV, SGRP] per s-group)
    and the kernel DMAs it instead of running 64 gpsimd local_scatters:
    the Pool engine (47us scatters + 11us drains per rep) drops to zero.
  - S psum tiles are [128, 3, 512] (3 banks) so each ACT exp call covers
    1536 elements instead of 1024, amortizing the ~352-cycle ACT pipeline
    fill; EV accumulators shrink to 2 rotating full-bank tiles (the four
    s-blocks are processed in two half-phases of two blocks each), keeping
    total PSUM usage at exactly 8 banks.
  - phases run g-major: (h0,g0) (h1,g0) (h0,g1) (h1,g1); per phase the S^T
    matmuls are WOVEN with the EV matmuls of the previous phase so the PE
    alternates between ACT-gated S work and dependency-free EV work.
"""

import sys

sys.path.insert(0, "/opt/trn_rl_repo")

import numpy as np

import concourse.bass as bass
import concourse.bacc as bacc
import concourse.mybir as mybir
import concourse.tile as tile
from concourse.vector_clock import ScopedClock

# ---------------------------------------------------------------------------
# Patch: this walrus build encodes at most ONE sync-wait on a CTRL NO_STRUCT
# instruction; TileContext's tail drain carries one wait per live proc.  Split
# the waits across a chain of single-wait drains.
# ---------------------------------------------------------------------------


def _patched_drain_and_barrier(self, tick_clock, wait_clock):
    drain_inst = self.nc.sync.drain()
    wait_clock.add_sem_waits(
        drain_inst.ins, ScopedClock({None: tick_clock.global_clock})
    )
    si = drain_inst.ins.sync_info
    if si is not None and len(si.on_wait) > 1:
        waits = list(si.on_wait)
        drain_inst.ins.sync_info = mybir.SyncInfo(
            on_wait=waits[:1], on_update=list(si.on_update)
        )
        for i in range(1, len(waits)):
            extra = self.nc.sync.drain()
            extra.ins.sync_info = mybir.SyncInfo(on_wait=[waits[i]], on_update=[])
    self.nc.all_engine_barrier()
    assert self.sems is not None
    popped = self.nc._tile_sem_poison_stack.pop()
    assert popped is self._sem_poison
    self.nc.clear_and_free_semaphores(list(self.sems.allocated().values()))
    self.nc.all_engine_barrier()


tile.TileContext._drain_and_barrier = _patched_drain_and_barrier

FP = mybir.dt.float32
BF = mybir.dt.bfloat16


class Cfg:
    def __init__(self, HPC=2, SQ=1024, SKV=4096, D=128, TOPK=64):
        self.HPC = HPC  # heads per core
        self.SQ = SQ
        self.SKV = SKV
        self.D = D
        self.TOPK = TOPK
        self.NKV = SKV // 128  # kv chunks of 128
        self.NSB = SQ // 128  # query blocks of 128
        self.SHALF = 512  # s-group width (s-dim per group)
        self.scale = float(D) ** -0.5


# ---------------------------------------------------------------------------
# Program builder
# ---------------------------------------------------------------------------


def build_program(cfg, nmaxs=None, reps=1):
    nc = bacc.Bacc("TRN2", debug=False)
    HPC, SQ, SKV, D, NKV = cfg.HPC, cfg.SQ, cfg.SKV, cfg.D, cfg.NKV
    NGRP = SQ // cfg.SHALF

    qT = nc.dram_tensor("qT", [HPC, D, SQ], BF, kind="ExternalInput").ap()
    kT = nc.dram_tensor("kT", [HPC, D, SKV], BF, kind="ExternalInput").ap()
    v = nc.dram_tensor("v", [HPC, SKV, D], BF, kind="ExternalInput").ap()
    ts = nc.dram_tensor(
        "ts", [NGRP, 128, NKV, cfg.SHALF], BF, kind="ExternalInput"
    ).ap()
    out = nc.dram_tensor("out", [HPC, SQ, D], FP, kind="ExternalOutput").ap()

    with tile.TileContext(nc) as tc:
        import contextlib

        ctx = contextlib.ExitStack()
        with ctx:
            tst_pool = ctx.enter_context(tc.tile_pool(name="tst", bufs=2))
            ktr_pool = ctx.enter_context(tc.tile_pool(name="ktr", bufs=2))
            et_pool = ctx.enter_context(tc.tile_pool(name="et", bufs=2))
            small_pool = ctx.enter_context(tc.tile_pool(name="small", bufs=4))
            out_pool = ctx.enter_context(tc.tile_pool(name="outp", bufs=4))
            s_psum = ctx.enter_context(tc.tile_pool(name="sps", bufs=2, space="PSUM"))
            ev_psum = ctx.enter_context(tc.tile_pool(name="evp", bufs=2, space="PSUM"))

            def _body(_iv=None):
                _build_body(
                    nc, tc, cfg, qT, kT, v, ts, out,
                    tst_pool, ktr_pool, et_pool, small_pool, out_pool,
                    s_psum, ev_psum,
                )

            if reps == 1:
                _body()
            else:
                with tc.For_i(
                    0, reps, 1,
                    hint_engines=(
                        mybir.EngineType.PE,
                        mybir.EngineType.DVE,
                        mybir.EngineType.Activation,
                        mybir.EngineType.Pool,
                        mybir.EngineType.SP,
                    ),
                ):
                    _body()

    nc.compile()
    return nc


def _build_body(nc, tc, cfg, qT, kT, v, ts, out,
                tst_pool, ktr_pool, et_pool, small_pool, out_pool,
                s_psum, ev_psum):
    HPC, SQ, SKV, D, NKV = cfg.HPC, cfg.SQ, cfg.SKV, cfg.D, cfg.NKV
    SGRP = cfg.SHALF
    NGRP = SQ // SGRP
    NSBG = SGRP // 128  # s-blocks per group (4)
    STG = 16            # kv-chunks per v staging DMA

    # ---------------- input DMAs (contiguous, host-prepped layouts) ---------
    def _load_head(h):
        qtr = ktr_pool.tile([128, SQ], BF, tag="qtr")
        nc.sync.dma_start(qtr[:], qT[h])
        ktr = ktr_pool.tile([128, SKV], BF, tag="ktr")
        nc.sync.dma_start(ktr[:, 0 : SKV // 2], kT[h, :, 0 : SKV // 2])
        nc.sync.dma_start(ktr[:, SKV // 2 : SKV], kT[h, :, SKV // 2 : SKV])
        vaug = ktr_pool.tile([128, NKV, D + 1], BF, tag="vaug")
        vview = v[h].rearrange("(n p) d -> p n d", p=128)
        for gdma in range(NKV // STG):
            nc.sync.dma_start(
                vaug[:, gdma * STG : (gdma + 1) * STG, 0:D],
                vview[:, gdma * STG : (gdma + 1) * STG, :],
            )
        nc.vector.memset(vaug[:, :, D : D + 1], 1.0)
        return qtr, ktr, vaug

    tst0 = tst_pool.tile([128, NKV, SGRP], BF, tag="tst")
    nc.sync.dma_start(tst0[:], ts[0])
    qtr0, ktr0, vaug0 = _load_head(0)
    qtr1, ktr1, vaug1 = _load_head(1)
    tst1 = tst_pool.tile([128, NKV, SGRP], BF, tag="tst")
    nc.sync.dma_start(tst1[:], ts[1])
    qtrs, ktrs, vaugs = [qtr0, qtr1], [ktr0, ktr1], [vaug0, vaug1]
    tsts = [tst0, tst1]

    # ---------------- woven compute phases ----------------------------------
    phases = [(h, g) for g in range(NGRP) for h in range(HPC)]

    class EvState:
        """Pending EV accumulation for one finished phase.  The four
        s-blocks advance in two half-phases of two blocks each; within a
        half-phase the two J-inner accumulation streams alternate between
        two rotating full-bank PSUM tiles so consecutive matmuls never
        target the same bank (avoids the accumulate RMW stall); each block
        is normalized and stored as soon as its J-loop closes."""

        def __init__(self, h, g, et):
            self.h, self.g, self.et = h, g, et
            self.seq = [
                (2 * half + i, J)
                for half in range(2)
                for J in range(NKV)
                for i in range(2)
            ]
            self.pos = 0
            self.ops = [None] * NSBG

        def emit(self, n):
            vaug = vaugs[self.h]
            for _ in range(n):
                if self.pos >= len(self.seq):
                    return
                b, J = self.seq[self.pos]
                if J == 0:
                    self.ops[b] = ev_psum.tile(
                        [128, 512], FP, tag="evacc", name="evacc"
                    )
                nc.tensor.matmul(
                    self.ops[b][:, 0 : D + 1],
                    self.et[:, J, b * 128 : (b + 1) * 128],
                    vaug[:, J, :],
                    start=(J == 0), stop=(J == NKV - 1),
                )
                if J == NKV - 1:
                    sb = self.g * NSBG + b
                    recip = small_pool.tile([128, 1], FP, tag="recip")
                    nc.vector.reciprocal(recip[:], self.ops[b][:, D : D + 1])
                    ot = out_pool.tile([128, D], FP, tag="ot")
                    nc.vector.tensor_scalar_mul(ot[:], self.ops[b][:, 0:D], recip[:])
                    nc.sync.dma_start(
                        out[self.h, sb * 128 : (sb + 1) * 128, :], ot[:]
                    )
                self.pos += 1

        def finish(self):
            self.emit(len(self.seq) - self.pos)

    TRIP = 3
    groups = [TRIP] * (NKV // TRIP) + ([NKV % TRIP] if NKV % TRIP else [])
    nweave = (2 * NKV + len(groups) - 1) // len(groups)  # EV matmuls per S-group

    prev = None  # EvState of the phase whose EV is pending
    for pi, (h, g) in enumerate(phases):
        qtr, ktr, tst = qtrs[h], ktrs[h], tsts[g]
        et = et_pool.tile([128, NKV, SGRP], BF, tag="et")
        sl = slice(g * SGRP, (g + 1) * SGRP)
        J = 0
        for w in groups:
            sp = s_psum.tile([128, TRIP, SGRP], FP, tag="sps")
            for t in range(w):
                nc.tensor.matmul(
                    sp[:, t, :],
                    ktr[:, (J + t) * 128 : (J + t + 1) * 128],
                    qtr[:, sl],
                    start=True, stop=True,
                )
            # weave: EV matmuls of the previous phase
            if prev is not None:
                prev.emit(nweave)
            nc.scalar.activation(
                et[:, J : J + w, :], sp[:, 0:w, :],
                mybir.ActivationFunctionType.Exp,
            )
            nc.vector.tensor_mul(
                et[:, J : J + w, :], et[:, J : J + w, :], tst[:, J : J + w, :]
            )
            J += w
        if prev is not None:
            prev.finish()
        prev = EvState(h, g, et)

    # tail: EV of the last phase runs unwoven (next rep's prep overlaps it)
    if prev is not None:
        prev.finish()


# ---------------------------------------------------------------------------
# Entry point: full unsharded inputs -> full output.
# Sharding: head-parallel, 2 heads per NeuronCore across 8 cores; the
# topk index/score tensors are shared by all cores.
# ---------------------------------------------------------------------------

_CACHE = {}


def make_in_maps(q, k, v, topk_indices, topk_scores, cfg):
    """Host-side prep: bf16 conversion, q pre-scaling + transpose, k
    transpose, dense TS table build.  Returns (in_maps, nmaxs)."""
    import ml_dtypes

    bf16 = ml_dtypes.bfloat16
    SQ, SKV, NKV, SGRP = cfg.SQ, cfg.SKV, cfg.NKV, cfg.SHALF
    NGRP = SQ // SGRP

    # dense TS[j, s] = sum of topk_scores over duplicate (s, j) selections
    idx = np.asarray(topk_indices)[0].astype(np.int64)          # [SQ, TOPK]
    sc = np.asarray(topk_scores, dtype=np.float32)[0]           # [SQ, TOPK]
    tsd = np.zeros((SKV, SQ), dtype=np.float32)                 # [j, s]
    s_arr = np.repeat(np.arange(SQ, dtype=np.int64), cfg.TOPK)
    np.add.at(tsd, (idx.reshape(-1), s_arr), sc.reshape(-1))
    # per group: [128, NKV, SGRP] with ts[p, J, s] = tsd[J*128 + p, g*SGRP + s]
    tsd = tsd.reshape(NKV, 128, NGRP, SGRP).transpose(2, 1, 0, 3)  # [g,p,J,s]
    ts_bf = np.ascontiguousarray(tsd.astype(bf16))

    qs = (np.asarray(q, dtype=np.float32) * (float(cfg.D) ** -0.5)).astype(bf16)
    kb = np.asarray(k, dtype=np.float32).astype(bf16)
    vb = np.asarray(v, dtype=np.float32).astype(bf16)
    qsT = np.ascontiguousarray(qs[0].transpose(0, 2, 1))  # [H, D, SQ]
    kbT = np.ascontiguousarray(kb[0].transpose(0, 2, 1))  # [H, D, SKV]

    in_maps = []
    for i in range(8):
        m = {
            "qT": np.ascontiguousarray(qsT[2 * i : 2 * i + 2]),
            "kT": np.ascontiguousarray(kbT[2 * i : 2 * i + 2]),
            "v": np.ascontiguousarray(vb[0, 2 * i : 2 * i + 2]),
            "ts": ts_bf,
        }
        in_maps.append(m)
    return in_maps, ()


def kernel(q, k, v, topk_indices, topk_scores):
    q = np.asarray(q, dtype=np.float32)
    B, H, SQ, D = q.shape
    SKV = np.asarray(k).shape[2]
    TOPK = np.asarray(topk_indices).shape[-1]
    assert B == 1 and H == 16 and SQ == 1024 and SKV == 4096 and D == 128

    cfg = Cfg(HPC=H // 8, SQ=SQ, SKV=SKV, D=D, TOPK=TOPK)
    in_maps, nmaxs = make_in_maps(q, k, v, topk_indices, topk_scores, cfg)

    nc = _CACHE.get("v3")
    if nc is None:
        nc = build_program(cfg, list(nmaxs), reps=1)
        _CACHE["v3"] = nc

    from concourse.bass_utils import run_bass_kernel_spmd

    res = run_bass_kernel_spmd(nc, in_maps, list(range(8)))
    out = np.stack([res.results[i]["out"] for i in range(8)])
    return out.reshape(1, H, SQ, D).astype(np.float32)
